# revision 3
# baseline (speedup 1.0000x reference)
"""DCT2D kernel for Trainium2 (8 NeuronCores, SPMD data-parallel).

Math: per 8x8 block  out = scale * (C^T (x - 128) C)
  == flat form:  out_flat[n, uv] = sum_xy (x_flat[n, xy] - 128) * T[xy, uv] * s[uv]
  == x_flat @ W + bias        with W[xy, uv] = T[xy, uv] * s[uv],
                                   bias[uv]  = -128 * sum_xy W[xy, uv]

Device-side layout trick: the PE contracts over the partition dim, so the
host pre-transposes each core's shard to [128, R/2] -- two consecutive
blocks stacked on partitions (block 2f on partitions 0..63, block 2f+1 on
64..127) -- and the weights become blockdiag(W, W) [128, 128].  One fp32
matmul per 512 columns (PSUM bank limit), then the PSUM->SBUF move is a
fused per-partition bias add (DVE) written back IN PLACE into the input
tile, then a straight DMA out in the same packed layout (issued on the
scalar-engine HWDGE ring so it doesn't FIFO behind the input DMAs on the
sync ring).  The host undoes the packing.  DRAM tensors are tile-major
[ntiles, 128, tile_f] so each 4 MiB DMA touches one contiguous HBM
extent (measured ~6 us/pass better than a [128, RP] row-major layout
whose transfers scatter 128x 32 KiB extents).  Measured ~152-164
us/core steady state (machine-load dependent) vs a ~141 us HBM roofline
(50.3 MB traffic @ 358 GB/s); equals the pure DMA-copy floor.
"""

import sys

if "/opt/trn_rl_repo" not in sys.path:
    sys.path.insert(0, "/opt/trn_rl_repo")

import numpy as np

import concourse.bass as bass  # noqa: F401
import concourse.mybir as mybir
import concourse.tile as tile
from concourse import bacc
from concourse.bass_utils import run_bass_kernel_spmd

N_CORES = 8
BLOCK = 8
B_DIM = 262144
C_DIM = 3
NBLK = B_DIM * C_DIM          # 786432 total 8x8 blocks
R = NBLK // N_CORES           # 98304 blocks per core
RP = R // 2                   # 49152 packed columns per core
TILE_F = 8192                 # columns per SBUF tile (4 MiB per DMA)
MM_F = 512                    # columns per matmul (one PSUM bank, fp32)

_CACHE = {}
last_results = None  # BassKernelResults of the most recent run (for test harness)


def _emit_pass(nc, xpool, pspool, w_sb, b_sb, xt, out_t, rp, tile_f):
    """One full pass: xt (DRAM, tile-major [nt,128,tile_f]) -> dct+bias -> out_t."""
    f32 = mybir.dt.float32
    for t in range(rp // tile_f):
        xin = xpool.tile([128, tile_f], f32)
        nc.sync.dma_start(xin[:], xt[t])
        for j in range(tile_f // MM_F):
            ps = pspool.tile([128, MM_F], f32)
            nc.tensor.matmul(
                ps[:], w_sb[:], xin[:, j * MM_F : (j + 1) * MM_F],
                start=True, stop=True,
            )
            nc.vector.tensor_scalar_add(
                xin[:, j * MM_F : (j + 1) * MM_F], ps[:], b_sb[:]
            )
        nc.scalar.dma_start(out_t[t], xin[:])


def _build_nc(rp=RP, tile_f=TILE_F):
    f32 = mybir.dt.float32
    nt = rp // tile_f
    nc = bacc.Bacc(None, target_bir_lowering=False, debug=False)
    xt = nc.declare_dram_parameter("xt", [nt, 128, tile_f], f32, isOutput=False)
    w = nc.declare_dram_parameter("w", [128, 128], f32, isOutput=False)
    bv = nc.declare_dram_parameter("bv", [128, 1], f32, isOutput=False)
    out = nc.declare_dram_parameter("out", [nt, 128, tile_f], f32, isOutput=True)

    with tile.TileContext(nc) as tc:
        with (
            tc.tile_pool(name="consts", bufs=1) as cpool,
            tc.tile_pool(name="xin", bufs=6) as xpool,
            tc.tile_pool(name="ps", bufs=8, space="PSUM") as pspool,
        ):
            w_sb = cpool.tile([128, 128], f32)
            nc.sync.dma_start(w_sb[:], w[:])
            b_sb = cpool.tile([128, 1], f32)
            nc.sync.dma_start(b_sb[:], bv[:])
            _emit_pass(nc, xpool, pspool, w_sb, b_sb, xt, out, rp, tile_f)
    nc.compile()
    return nc


def _build_bench(nrep, rp=RP, tile_f=TILE_F):
    """Bench variant: loop the steady-state pass `nrep` times over Internal
    DRAM scratch (no host I/O).  Used by bench2.measure_hw_ns."""
    f32 = mybir.dt.float32
    nt = rp // tile_f
    nc = bacc.Bacc(None, target_bir_lowering=False, debug=False)
    w = nc.declare_dram_parameter("w", [128, 128], f32, isOutput=False)
    bv = nc.declare_dram_parameter("bv", [128, 1], f32, isOutput=False)
    dummy = nc.declare_dram_parameter("bench_out", [128, 8], f32, isOutput=True)
    xt = nc.dram_tensor("xbench", [nt, 128, tile_f], f32)
    out = nc.dram_tensor("obench", [nt, 128, tile_f], f32)

    with tile.TileContext(nc) as tc:
        with (
            tc.tile_pool(name="consts", bufs=1) as cpool,
            tc.tile_pool(name="xin", bufs=6) as xpool,
            tc.tile_pool(name="ps", bufs=8, space="PSUM") as pspool,
        ):
            w_sb = cpool.tile([128, 128], f32)
            nc.sync.dma_start(w_sb[:], w[:])
            b_sb = cpool.tile([128, 1], f32)
            nc.sync.dma_start(b_sb[:], bv[:])
            for _ in range(nrep):
                _emit_pass(nc, xpool, pspool, w_sb, b_sb, xt, out, rp, tile_f)
            nc.sync.dma_start(dummy[:], w_sb[:, :8])
    nc.compile()
    in_maps = [
        {"w": np.zeros((128, 128), np.float32), "bv": np.zeros((128, 1), np.float32)}
        for _ in range(N_CORES)
    ]
    return nc, in_maps


def _consts(dct_tensor, scale):
    t_flat = np.asarray(dct_tensor, dtype=np.float64).reshape(64, 64)
    s_flat = np.asarray(scale, dtype=np.float64).reshape(64)
    w64 = (t_flat * s_flat[None, :]).astype(np.float32)
    w = np.zeros((128, 128), dtype=np.float32)
    w[:64, :64] = w64
    w[64:, 64:] = w64
    bias = (-128.0 * w.astype(np.float64).sum(axis=0)).astype(np.float32)
    return w, bias.reshape(128, 1)


def kernel(x, dct_tensor, scale):
    w, bias = _consts(dct_tensor, scale)

    from concurrent.futures import ThreadPoolExecutor

    nt = RP // TILE_F
    xf = np.ascontiguousarray(np.asarray(x, dtype=np.float32)).reshape(NBLK, 64)

    def _pack(c):
        shard = xf[c * R : (c + 1) * R]
        # xt[t, p*64+k, f] = shard[2*(t*TILE_F+f)+p, k]
        return np.ascontiguousarray(
            shard.reshape(nt, TILE_F, 2, 64).transpose(0, 2, 3, 1)
        ).reshape(nt, 128, TILE_F)

    with ThreadPoolExecutor(N_CORES) as pool:
        packs = list(pool.map(_pack, range(N_CORES)))
    in_maps = [{"xt": p, "w": w, "bv": bias} for p in packs]

    if "nc" not in _CACHE:
        _CACHE["nc"] = _build_nc()
    res = run_bass_kernel_spmd(_CACHE["nc"], in_maps, core_ids=list(range(N_CORES)))
    global last_results
    last_results = res

    full = np.empty((NBLK, 64), dtype=np.float32)

    def _unpack(c):
        o = np.asarray(res.results[c]["out"])  # [nt, 128, TILE_F] packed
        full[c * R : (c + 1) * R] = (
            o.reshape(nt, 2, 64, TILE_F).transpose(0, 3, 1, 2).reshape(R, 64)
        )

    with ThreadPoolExecutor(N_CORES) as pool:
        list(pool.map(_unpack, range(N_CORES)))
    return full.reshape(B_DIM, C_DIM, BLOCK, BLOCK)



# revision 5
# speedup vs baseline: 2.2189x; 2.2189x over previous
"""DCT2D kernel for Trainium2 (8 NeuronCores, SPMD data-parallel).

Math: per 8x8 block  out = scale * (C^T (x - 128) C)
  == flat form:  out_flat[n, uv] = sum_xy (x_flat[n, xy] - 128) * T[xy, uv] * s[uv]
  == (x_flat - 128) @ W        with W[xy, uv] = T[xy, uv] * s[uv]

The problem is HBM-bound (tiny weights, streaming data), so I/O dtype is
the lever: the host pre-centers x by 128 (exact in fp32) and casts to
fp16, the device computes fp16 matmuls into fp32 PSUM, and the output is
downcast to fp16 on the way out -- 25.2 MB/core of traffic instead of
the 50.3 MB a fp32 kernel moves (rel err ~5e-4, well inside the 2e-2
gate).

Device-side layout: the PE contracts over the partition dim, so the host
pre-transposes each core's shard to [128, R/2] -- two consecutive blocks
stacked on partitions (block 2f on partitions 0..63, block 2f+1 on
64..127) -- and the weights become blockdiag(W, W) [128, 128].  One fp16
matmul per 512 columns (PSUM bank limit), then the fp32->fp16
PSUM->SBUF downcast is round-robined across the Act, Pool and DVE
engines (no single engine can keep up with the ~69 us/pass DMA floor on
its own).  Input DMAs ride the sync-engine ring, output DMAs the
scalar-engine ring so the two directions don't FIFO behind each other.
DRAM tensors are tile-major [ntiles, 128, tile_f] so each DMA touches
one contiguous HBM extent.  The host undoes the packing.
"""

import sys

if "/opt/trn_rl_repo" not in sys.path:
    sys.path.insert(0, "/opt/trn_rl_repo")

import numpy as np

import concourse.bass as bass  # noqa: F401
import concourse.mybir as mybir
import concourse.tile as tile
from concourse import bacc
from concourse.bass_utils import run_bass_kernel_spmd

N_CORES = 8
BLOCK = 8
B_DIM = 262144
C_DIM = 3
NBLK = B_DIM * C_DIM          # 786432 total 8x8 blocks
R = NBLK // N_CORES           # 98304 blocks per core
RP = R // 2                   # 49152 packed columns per core
TILE_F = 8192                 # columns per SBUF tile (2 MiB per DMA in fp16)
MM_F = 512                    # columns per matmul (one PSUM bank, fp32)

_CACHE = {}
last_results = None  # BassKernelResults of the most recent run (for test harness)


def _emit_pass(nc, xpool, opool, pspool, w_sb, xt, out_t, rp, tile_f):
    """One full pass: xt (DRAM fp16, tile-major [nt,128,tile_f]) -> dct -> out_t."""
    f16 = mybir.dt.float16
    f32 = mybir.dt.float32
    # PSUM->SBUF downcast engine schedule: GPSIMD/Pool cannot access PSUM
    # on TRN2, so split across Act (1.2 GHz) and DVE (0.96 GHz), 9/7.
    def _copy_scalar(dst, src):
        nc.scalar.copy(dst, src)

    def _copy_vector(dst, src):
        nc.vector.tensor_copy(dst, src)

    copies = [_copy_scalar, _copy_vector] * 7 + [_copy_scalar, _copy_scalar]
    for t in range(rp // tile_f):
        xin = xpool.tile([128, tile_f], f16)
        nc.sync.dma_start(xin[:], xt[t])
        osb = opool.tile([128, tile_f], f16)
        for j in range(tile_f // MM_F):
            ps = pspool.tile([128, MM_F], f32)
            nc.tensor.matmul(
                ps[:], w_sb[:], xin[:, j * MM_F : (j + 1) * MM_F],
                start=True, stop=True,
            )
            copies[j % 16](osb[:, j * MM_F : (j + 1) * MM_F], ps[:])
        nc.scalar.dma_start(out_t[t], osb[:])


def _build_nc(rp=RP, tile_f=TILE_F):
    f16 = mybir.dt.float16
    nt = rp // tile_f
    nc = bacc.Bacc(None, target_bir_lowering=False, debug=False)
    xt = nc.declare_dram_parameter("xt", [nt, 128, tile_f], f16, isOutput=False)
    w = nc.declare_dram_parameter("w", [128, 128], f16, isOutput=False)
    out = nc.declare_dram_parameter("out", [nt, 128, tile_f], f16, isOutput=True)

    with tile.TileContext(nc) as tc:
        with (
            tc.tile_pool(name="consts", bufs=1) as cpool,
            tc.tile_pool(name="xin", bufs=4) as xpool,
            tc.tile_pool(name="osb", bufs=4) as opool,
            tc.tile_pool(name="ps", bufs=8, space="PSUM") as pspool,
        ):
            w_sb = cpool.tile([128, 128], f16)
            nc.sync.dma_start(w_sb[:], w[:])
            _emit_pass(nc, xpool, opool, pspool, w_sb, xt, out, rp, tile_f)
    nc.compile()
    return nc


def _build_bench(nrep, rp=RP, tile_f=TILE_F):
    """Bench variant: loop the steady-state pass `nrep` times over Internal
    DRAM scratch (no host I/O).  Used by bench2.measure_hw_ns."""
    f16 = mybir.dt.float16
    f32 = mybir.dt.float32
    nt = rp // tile_f
    nc = bacc.Bacc(None, target_bir_lowering=False, debug=False)
    w = nc.declare_dram_parameter("w", [128, 128], f16, isOutput=False)
    dummy = nc.declare_dram_parameter("bench_out", [128, 8], f32, isOutput=True)
    xt = nc.dram_tensor("xbench", [nt, 128, tile_f], f16)
    out = nc.dram_tensor("obench", [nt, 128, tile_f], f16)

    with tile.TileContext(nc) as tc:
        with (
            tc.tile_pool(name="consts", bufs=1) as cpool,
            tc.tile_pool(name="xin", bufs=4) as xpool,
            tc.tile_pool(name="osb", bufs=4) as opool,
            tc.tile_pool(name="ps", bufs=8, space="PSUM") as pspool,
        ):
            w_sb = cpool.tile([128, 128], f16)
            nc.sync.dma_start(w_sb[:], w[:])
            dsb = cpool.tile([128, 8], f32)
            nc.vector.tensor_copy(dsb[:], w_sb[:, :8])
            for _ in range(nrep):
                _emit_pass(nc, xpool, opool, pspool, w_sb, xt, out, rp, tile_f)
            nc.sync.dma_start(dummy[:], dsb[:])
    nc.compile()
    in_maps = [{"w": np.zeros((128, 128), np.float16)} for _ in range(N_CORES)]
    return nc, in_maps


def _consts(dct_tensor, scale):
    t_flat = np.asarray(dct_tensor, dtype=np.float64).reshape(64, 64)
    s_flat = np.asarray(scale, dtype=np.float64).reshape(64)
    w64 = t_flat * s_flat[None, :]
    w = np.zeros((128, 128), dtype=np.float64)
    w[:64, :64] = w64
    w[64:, 64:] = w64
    return w.astype(np.float16)


def kernel(x, dct_tensor, scale):
    w = _consts(dct_tensor, scale)

    from concurrent.futures import ThreadPoolExecutor

    nt = RP // TILE_F
    xf = np.ascontiguousarray(np.asarray(x, dtype=np.float32)).reshape(NBLK, 64)

    def _pack(c):
        shard = xf[c * R : (c + 1) * R]
        # xt[t, p*64+k, f] = shard[2*(t*TILE_F+f)+p, k] - 128
        a = shard.reshape(nt, TILE_F, 2, 64).transpose(0, 2, 3, 1)
        return (a - np.float32(128.0)).astype(np.float16).reshape(nt, 128, TILE_F)

    with ThreadPoolExecutor(N_CORES) as pool:
        packs = list(pool.map(_pack, range(N_CORES)))
    in_maps = [{"xt": p, "w": w} for p in packs]

    if "nc" not in _CACHE:
        _CACHE["nc"] = _build_nc()
    res = run_bass_kernel_spmd(_CACHE["nc"], in_maps, core_ids=list(range(N_CORES)))
    global last_results
    last_results = res

    full = np.empty((NBLK, 64), dtype=np.float32)

    def _unpack(c):
        o = np.asarray(res.results[c]["out"])  # fp16 [nt, 128, TILE_F] packed
        dst = full[c * R : (c + 1) * R].reshape(nt, TILE_F, 2, 64)
        dst[:] = o.reshape(nt, 2, 64, TILE_F).transpose(0, 3, 1, 2)

    with ThreadPoolExecutor(N_CORES) as pool:
        list(pool.map(_unpack, range(N_CORES)))
    return full.reshape(B_DIM, C_DIM, BLOCK, BLOCK)


# revision 7
# speedup vs baseline: 2.6816x; 1.2085x over previous
"""DCT2D kernel for Trainium2 (8 NeuronCores, SPMD data-parallel).

Math: per 8x8 block  out = scale * (C^T (x - 128) C)
  == flat form:  out_flat[n, uv] = sum_xy (x_flat[n, xy] - 128) * T[xy, uv] * s[uv]
  == (x_flat - 128) @ W        with W[xy, uv] = T[xy, uv] * s[uv]

The problem is HBM-bound (tiny weights, streaming data), so I/O dtype is
the lever: the host pre-centers x by 128 (exact in fp32) and casts to
fp16, the device computes fp16 matmuls into fp32 PSUM, and the output is
downcast to fp16 on the way out -- 25.2 MB/core of traffic instead of
the 50.3 MB a fp32 kernel moves (rel err ~5e-4, well inside the 2e-2
gate).

Device-side layout: the PE contracts over the partition dim, so the host
pre-transposes each core's shard to [128, R/2] -- two consecutive blocks
stacked on partitions (block 2f on partitions 0..63, block 2f+1 on
64..127) -- and the weights become blockdiag(W, W) [128, 128].  One fp16
matmul per 512 columns (PSUM bank limit), then the fp32->fp16
PSUM->SBUF downcast is round-robined across the Act, Pool and DVE
engines (no single engine can keep up with the ~69 us/pass DMA floor on
its own).  Input DMAs ride the sync-engine ring, output DMAs the
scalar-engine ring so the two directions don't FIFO behind each other.
DRAM tensors are tile-major [ntiles, 128, tile_f] so each DMA touches
one contiguous HBM extent.  The host undoes the packing.
"""

import sys

if "/opt/trn_rl_repo" not in sys.path:
    sys.path.insert(0, "/opt/trn_rl_repo")

import numpy as np

import concourse.bass as bass  # noqa: F401
import concourse.mybir as mybir
import concourse.tile as tile
from concourse import bacc
from concourse.bass_utils import run_bass_kernel_spmd

N_CORES = 8
BLOCK = 8
B_DIM = 262144
C_DIM = 3
NBLK = B_DIM * C_DIM          # 786432 total 8x8 blocks
R = NBLK // N_CORES           # 98304 blocks per core
RP = R // 2                   # 49152 packed columns per core
TILE_F = 8192                 # columns per SBUF tile (2 MiB per DMA in fp16)
MM_F = 512                    # columns per matmul (one PSUM bank, fp32)

_CACHE = {}
last_results = None  # BassKernelResults of the most recent run (for test harness)


def _emit_pass(nc, xpool, opool, pspool, w_sb, xt, out_t, rp, tile_f):
    """One full pass: xt (DRAM fp16, tile-major [nt,128,tile_f]) -> dct -> out_t."""
    f16 = mybir.dt.float16
    f32 = mybir.dt.float32
    # PSUM->SBUF downcast engine schedule: GPSIMD/Pool cannot access PSUM
    # on TRN2, so split across Act (1.2 GHz) and DVE (0.96 GHz), 5/3 of the
    # bank-pair copies.  Each copy spans two adjacent PSUM banks (1024
    # cols) to halve the instruction/semaphore count.  Output DMAs ride
    # the otherwise-idle Pool-engine ring; input DMAs the sync ring.
    def _copy_scalar(dst, src):
        nc.scalar.copy(dst, src)

    def _copy_vector(dst, src):
        nc.vector.tensor_copy(dst, src)

    copies = [_copy_scalar, _copy_vector] * 3 + [_copy_scalar, _copy_scalar]
    for t in range(rp // tile_f):
        xin = xpool.tile([128, tile_f], f16)
        nc.sync.dma_start(xin[:], xt[t])
        osb = opool.tile([128, tile_f], f16)
        for j in range(tile_f // (2 * MM_F)):
            ps = pspool.tile([128, 2 * MM_F], f32)
            for h in range(2):
                lo = (2 * j + h) * MM_F
                nc.tensor.matmul(
                    ps[:, h * MM_F : (h + 1) * MM_F],
                    w_sb[:], xin[:, lo : lo + MM_F],
                    start=True, stop=True,
                )
            copies[j % 8](osb[:, 2 * j * MM_F : (2 * j + 2) * MM_F], ps[:])
        nc.gpsimd.dma_start(out_t[t], osb[:])


def _build_nc(rp=RP, tile_f=TILE_F):
    f16 = mybir.dt.float16
    nt = rp // tile_f
    nc = bacc.Bacc(None, target_bir_lowering=False, debug=False)
    xt = nc.declare_dram_parameter("xt", [nt, 128, tile_f], f16, isOutput=False)
    w = nc.declare_dram_parameter("w", [128, 128], f16, isOutput=False)
    out = nc.declare_dram_parameter("out", [nt, 128, tile_f], f16, isOutput=True)

    with tile.TileContext(nc) as tc:
        with (
            tc.tile_pool(name="consts", bufs=1) as cpool,
            tc.tile_pool(name="xin", bufs=4) as xpool,
            tc.tile_pool(name="osb", bufs=4) as opool,
            tc.tile_pool(name="ps", bufs=4, space="PSUM") as pspool,
        ):
            w_sb = cpool.tile([128, 128], f16)
            nc.sync.dma_start(w_sb[:], w[:])
            _emit_pass(nc, xpool, opool, pspool, w_sb, xt, out, rp, tile_f)
    nc.compile()
    return nc


def _build_bench(nrep, rp=RP, tile_f=TILE_F):
    """Bench variant: loop the steady-state pass `nrep` times over Internal
    DRAM scratch (no host I/O).  Used by bench2.measure_hw_ns."""
    f16 = mybir.dt.float16
    f32 = mybir.dt.float32
    nt = rp // tile_f
    nc = bacc.Bacc(None, target_bir_lowering=False, debug=False)
    w = nc.declare_dram_parameter("w", [128, 128], f16, isOutput=False)
    dummy = nc.declare_dram_parameter("bench_out", [128, 8], f32, isOutput=True)
    xt = nc.dram_tensor("xbench", [nt, 128, tile_f], f16)
    out = nc.dram_tensor("obench", [nt, 128, tile_f], f16)

    with tile.TileContext(nc) as tc:
        with (
            tc.tile_pool(name="consts", bufs=1) as cpool,
            tc.tile_pool(name="xin", bufs=4) as xpool,
            tc.tile_pool(name="osb", bufs=4) as opool,
            tc.tile_pool(name="ps", bufs=4, space="PSUM") as pspool,
        ):
            w_sb = cpool.tile([128, 128], f16)
            nc.sync.dma_start(w_sb[:], w[:])
            dsb = cpool.tile([128, 8], f32)
            nc.vector.tensor_copy(dsb[:], w_sb[:, :8])
            for _ in range(nrep):
                _emit_pass(nc, xpool, opool, pspool, w_sb, xt, out, rp, tile_f)
            nc.sync.dma_start(dummy[:], dsb[:])
    nc.compile()
    in_maps = [{"w": np.zeros((128, 128), np.float16)} for _ in range(N_CORES)]
    return nc, in_maps


def _consts(dct_tensor, scale):
    t_flat = np.asarray(dct_tensor, dtype=np.float64).reshape(64, 64)
    s_flat = np.asarray(scale, dtype=np.float64).reshape(64)
    w64 = t_flat * s_flat[None, :]
    w = np.zeros((128, 128), dtype=np.float64)
    w[:64, :64] = w64
    w[64:, 64:] = w64
    return w.astype(np.float16)


def kernel(x, dct_tensor, scale):
    w = _consts(dct_tensor, scale)

    from concurrent.futures import ThreadPoolExecutor

    nt = RP // TILE_F
    xf = np.ascontiguousarray(np.asarray(x, dtype=np.float32)).reshape(NBLK, 64)

    def _pack(c):
        shard = xf[c * R : (c + 1) * R]
        # xt[t, p*64+k, f] = shard[2*(t*TILE_F+f)+p, k] - 128
        a = shard.reshape(nt, TILE_F, 2, 64).transpose(0, 2, 3, 1)
        return (a - np.float32(128.0)).astype(np.float16).reshape(nt, 128, TILE_F)

    with ThreadPoolExecutor(N_CORES) as pool:
        packs = list(pool.map(_pack, range(N_CORES)))
    in_maps = [{"xt": p, "w": w} for p in packs]

    if "nc" not in _CACHE:
        _CACHE["nc"] = _build_nc()
    res = run_bass_kernel_spmd(_CACHE["nc"], in_maps, core_ids=list(range(N_CORES)))
    global last_results
    last_results = res

    full = np.empty((NBLK, 64), dtype=np.float32)

    def _unpack(c):
        o = np.asarray(res.results[c]["out"])  # fp16 [nt, 128, TILE_F] packed
        dst = full[c * R : (c + 1) * R].reshape(nt, TILE_F, 2, 64)
        dst[:] = o.reshape(nt, 2, 64, TILE_F).transpose(0, 3, 1, 2)

    with ThreadPoolExecutor(N_CORES) as pool:
        list(pool.map(_unpack, range(N_CORES)))
    return full.reshape(B_DIM, C_DIM, BLOCK, BLOCK)
